# revision 9
# baseline (speedup 1.0000x reference)
"""Trainium2 Bass kernel for nn_Attn_61366492725428 (masked attention pooling).

Reference computation:
    hid = transpose(hidden,(1,0,2)).reshape(B,-1)          # (B, 1024)
    e   = enc @ We + (hid @ Wh)[:,None] + b                # (B, T)
    e   = e * mask
    a   = softmax(e, axis=1) * mask;  a /= a.sum(1)
    ctx = einsum('bt,bth->bh', a, enc)                     # (B, 1024)

Key identity: the per-batch constant c = hid@Wh + b shifts every *valid*
energy equally, masked entries are zeroed in both the numerator and the
renormalization denominator, and softmax's own Z cancels under the
renormalize — so exp(c) cancels exactly and the output does not depend on
hidden/Wh/b at all:
    ctx[b] = sum_t mask*exp(e_enc) * enc / sum_t mask*exp(e_enc)
(verified vs the jax reference: rel err ~2e-6, pure fp noise).

Device pipeline per enc tile [128t, 1024h] (f32, natural layout):
    DVE : affine_mul_reduce -> p16 = fp16(enc*We), e[:,j] = sum_h (f32)
          (single fused pass; enc f32 tile freed immediately)
    PE  : ctx_psum += w16[t]^T @ p16      (fp16 matmul, 1 HW pass)
then ctx = ctx_psum * (1/S) * (1/We) — dividing the *product*-weighted sum
by We recovers the enc-weighted sum (error ~3e-4 from fp16 rounding of p).

Sharding: batch B=32 across 8 cores (4 batches/core); We replicated.
Host precomputes 1/We and the transposed mask layout (tiny arrays).
"""

import numpy as np

N_CORES = 8
B, T, HE = 32, 2048, 1024
B_LOC = B // N_CORES          # 4 batches per core
TT = 128                      # t-tile (partition dim)
NT = T // TT                  # 16 t-tiles per batch
NH = 512                      # matmul free-dim limit (one PSUM bank of f32)

_CACHE = {}


def _build_nc():
    import concourse.bacc as bacc
    import concourse.tile as tile
    from concourse import mybir

    f32 = mybir.dt.float32
    f16 = mybir.dt.float16
    Exp = mybir.ActivationFunctionType.Exp

    nc = bacc.Bacc("TRN2")
    enc = nc.dram_tensor("enc", [B_LOC, T, HE], f32, kind="ExternalInput")
    mskt = nc.dram_tensor("mskt", [128, B_LOC * NT], f32, kind="ExternalInput")
    we = nc.dram_tensor("we", [1, HE], f32, kind="ExternalInput")
    invwe = nc.dram_tensor("invwe", [1, HE], f32, kind="ExternalInput")
    out = nc.dram_tensor("out", [B_LOC, HE], f32, kind="ExternalOutput")

    with tile.TileContext(nc) as tc:
        with (
            tc.tile_pool(name="singles", bufs=1) as singles,
            tc.tile_pool(name="encp", bufs=12) as encp,
            tc.tile_pool(name="p16p", bufs=20) as p16p,
            tc.tile_pool(name="stats", bufs=4) as stats,
            tc.tile_pool(name="ctxp", bufs=2, space="PSUM") as ctxp,
            tc.tile_pool(name="spsum", bufs=2, space="PSUM") as spsum,
        ):
            # We broadcast to all 128 partitions via PE (ones[1,128]^T @ we[1,:]):
            # a K=1 matmul is ~3us vs ~26us for a 128x-replicated SWDGE DMA.
            we_row = singles.tile([1, HE], f32, tag="we_row")
            nc.sync.dma_start(out=we_row, in_=we[0:1, :])
            ones_row = singles.tile([1, 128], f32, tag="ones_row")
            nc.vector.memset(ones_row, 1.0)
            we_b = singles.tile([128, HE], f32, tag="we_b")
            with tc.tile_pool(name="bcast", bufs=1, space="PSUM") as bcastp:
                we_ps = bcastp.tile([128, 2, NH], f32, tag="we_ps")
                for h in range(2):
                    nc.tensor.matmul(
                        we_ps[:, h, :],
                        ones_row,
                        we_row[:, h * NH : (h + 1) * NH],
                        start=True,
                        stop=True,
                    )
                    nc.scalar.copy(we_b[:, h * NH : (h + 1) * NH], we_ps[:, h, :])

            inv_sb = singles.tile([1, HE], f32, tag="invwe")
            nc.sync.dma_start(out=inv_sb, in_=invwe[0:1, :])

            ones_col = singles.tile([128, 1], f32, tag="ones")
            nc.vector.memset(ones_col, 1.0)

            # transposed mask [t-within-tile, (b, tile)] — one natural DMA
            mask_all = singles.tile([128, B_LOC * NT], f32, tag="mask")
            nc.sync.dma_start(out=mask_all, in_=mskt[:, :])

            NC_ = 2            # softmax/matmul chunks per batch
            CS = NT // NC_     # tiles per chunk (8)
            for b in range(B_LOC):
                ctx = ctxp.tile([1, 2, NH], f32, tag="ctx")
                s_ps = spsum.tile([1, 1], f32, tag="s_ps")
                for c in range(NC_):
                    mb = mask_all[:, b * NT + c * CS : b * NT + (c + 1) * CS]
                    e_c = stats.tile([128, CS], f32, tag="e_c")
                    p16_tiles = []
                    for jj in range(CS):
                        j = c * CS + jj
                        et = encp.tile([128, HE], f32, tag="enc_t")
                        nc.sync.dma_start(
                            out=et, in_=enc[b, j * TT : (j + 1) * TT, :]
                        )
                        p16 = p16p.tile([128, HE], f16, tag="p16")
                        p16_tiles.append(p16)
                        # p16 = fp16(enc * We); e_c[:, jj] = sum_h enc*We (f32)
                        nc.vector.affine_mul_reduce(
                            out=p16,
                            accum_out=e_c[:, jj : jj + 1],
                            in0=et,
                            in1=we_b,
                            scale=1.0,
                            bias=0.0,
                        )

                    # w = mask * exp(e * mask); ws[p] = sum_jj w[p, jj]
                    masked = stats.tile([128, CS], f32, tag="masked")
                    nc.vector.tensor_mul(masked, e_c, mb)
                    expd = stats.tile([128, CS], f32, tag="expd")
                    nc.scalar.activation(expd, masked, Exp)
                    w_c = stats.tile([128, CS], f32, tag="w_c")
                    ws = stats.tile([128, 1], f32, tag="ws")
                    nc.vector.tensor_mul(w_c, expd, mb)
                    nc.vector.reduce_sum(ws, w_c, axis=mybir.AxisListType.X)
                    w16 = stats.tile([128, CS], f16, tag="w16")
                    nc.vector.tensor_copy(w16, w_c)

                    # S += sum_p ws[p]  (partition reduce via PE)
                    nc.tensor.matmul(
                        s_ps, ws, ones_col, start=(c == 0), stop=(c == NC_ - 1)
                    )

                    # ctxP[h] += sum_t w16[t] * p16[t, h]
                    for jj in range(CS):
                        for h in range(2):
                            nc.tensor.matmul(
                                ctx[:, h, :],
                                w16[:, jj : jj + 1],
                                p16_tiles[jj][:, h * NH : (h + 1) * NH],
                                start=(c == 0 and jj == 0),
                                stop=(c == NC_ - 1 and jj == CS - 1),
                            )

                recip = stats.tile([1, 1], f32, tag="recip")
                nc.vector.reciprocal(recip, s_ps)

                # out[b] = (ctxP * (1/S)) * (1/We)   — one fused DVE op
                ctx_sb = stats.tile([1, HE], f32, tag="ctx_sb")
                dummy = stats.tile([1, 1], f32, tag="dummy")
                nc.vector.affine_mul_reduce(
                    out=ctx_sb.rearrange("p (g h) -> p g h", g=2),
                    accum_out=dummy,
                    in0=ctx[:, :, :],
                    in1=inv_sb.rearrange("p (g h) -> p g h", g=2),
                    scale=recip,
                    bias=0.0,
                )
                nc.gpsimd.dma_start(out=out[b : b + 1, :], in_=ctx_sb)

    nc.compile()
    return nc


def _get_nc():
    if "nc" not in _CACHE:
        _CACHE["nc"] = _build_nc()
    return _CACHE["nc"]


def _prep_host_inputs(encoder_outputs, mask, W):
    enc = np.ascontiguousarray(np.asarray(encoder_outputs, dtype=np.float32))
    msk = np.asarray(mask, dtype=np.float32)
    we = np.ascontiguousarray(np.asarray(W, dtype=np.float32)[0:1, HE:])
    invwe = np.ascontiguousarray(1.0 / we)
    return enc, msk, we, invwe


def kernel(hidden, encoder_outputs, mask, W, b):
    from concourse import bass_utils

    # avoid S3 upload attempts if tracing is enabled
    bass_utils.upload_artifacts = lambda tmpdir: f"local:{tmpdir}"

    nc = _get_nc()
    enc, msk, we, invwe = _prep_host_inputs(encoder_outputs, mask, W)

    in_maps = []
    for i in range(N_CORES):
        mloc = msk[i * B_LOC : (i + 1) * B_LOC]               # [4, 2048]
        mskt = np.ascontiguousarray(
            mloc.reshape(B_LOC, NT, TT).transpose(2, 0, 1).reshape(TT, B_LOC * NT)
        )
        in_maps.append(
            {
                "enc": np.ascontiguousarray(enc[i * B_LOC : (i + 1) * B_LOC]),
                "mskt": mskt,
                "we": we,
                "invwe": invwe,
            }
        )

    def _run():
        return bass_utils.run_bass_kernel_spmd(
            nc, in_maps, core_ids=list(range(N_CORES))
        )

    try:
        res = _run()
    except Exception:
        # transient device-state failures have been observed; retry once
        res = _run()
    _CACHE["last_results"] = res
    return np.concatenate([r["out"] for r in res.results], axis=0)
